# revision 28
# baseline (speedup 1.0000x reference)
"""DGCN diffusion-graph-conv kernel for 8 Trainium2 NeuronCores.

Math (per the reference):
    support S = D^-1/2 (adj+I)^T D^-1/2  with D = diag(rowsum(adj+I))
    x_m = T_m(S) x0  (Chebyshev recurrence, K=3 -> m=0..3)
    out = sum_m x_m @ W_m + bias

Strategy (data-parallel over batch, 4 batches/core, mixed precision):
    Fold Chebyshev coefficients into the weights and expand the
    recurrence into explicit support powers (host-precomputed in fp32):
        V0 = W0 - W2, V1 = W1 - 3*W3, V2 = 2*W2, V3 = 4*W3
        U_m = x0 @ V_m                        (contracts feature dim)
        out = U0 + S*U1 + S^2*U2 + S^3*U3 + bias   (contracts nodes)
    U0 feeds the output undamped -> bf16 matmuls (full-rate, ~0.2% err).
    U1..U3 and the S^m multiplies are damped by the contractive support
    spectrum -> fp8 e4m3 DoubleRow matmuls (2x PE rate).  The three
    S^m terms accumulate into one PSUM group, so there is no serial
    diffusion chain on the device at all.
    Power-of-2 scales keep fp8 operands in the normal range:
        x*8, V_m*32 (m>=1), S^m*2^14, U_m carried *8.
    Host precomputes S, S^2, S^3, the V combos, all layout permutes and
    fp8 quantization.  Measured end-to-end rel err ~9e-3 (gate 2e-2).
"""

import numpy as np
import ml_dtypes

import concourse.bacc as bacc
import concourse.tile as tile
import concourse.mybir as mybir
from concourse.bass_utils import run_bass_kernel_spmd

F32 = mybir.dt.float32
BF16 = mybir.dt.bfloat16
F8 = mybir.dt.float8e4
DR = mybir.MatmulPerfMode.DoubleRow
ALU = mybir.AluOpType
AFT = mybir.ActivationFunctionType
E4M3 = ml_dtypes.float8_e4m3

N_CORES = 8
B, N, D = 32, 512, 768
BL = B // N_CORES          # local batches per core = 4
BN = BL * N                # local rows = 2048
NT = N // 128              # 4 node tiles per batch
DT = D // 128              # 6 feature k-subtiles
PW = 1536                  # batch-pair column width (2*768)

SX = 8.0                   # x fp8 pre-scale
SV = 32.0                  # V1..V3 fp8 pre-scale
SS = float(2 ** 14)        # S^m fp8 pre-scale
SU = 8.0                   # U_m carried *8 in fp8
C_U = SU / (SX * SV)       # psum(U123) -> 8*U_m     (= 1/32)
C_O = 1.0 / (SS * SU)      # psum(combine) -> out    (= 2^-17)


def _build_program():
    nc = bacc.Bacc("TRN2", target_bir_lowering=False, debug=False,
                   num_devices=N_CORES)
    # All inputs host-permuted to the exact SBUF tile layout
    # ([partition, ...free]) so every DMA moves full contiguous lines.
    x0R_d = nc.dram_tensor("x0R", [BN // 128, 128, DT, 128], BF16,
                           kind="ExternalInput").ap()
    x8P_d = nc.dram_tensor("x8P", [BN // 128, 128, DT, 128], F8,
                           kind="ExternalInput").ap()
    v0P_d = nc.dram_tensor("v0P", [128, DT, D], BF16,
                           kind="ExternalInput").ap()
    v8P_d = nc.dram_tensor("v8P", [3, 128, DT, D], F8,
                           kind="ExternalInput").ap()
    s8P_d = nc.dram_tensor("s8P", [3, 128, NT, N], F8,
                           kind="ExternalInput").ap()
    bias_d = nc.dram_tensor("bias", [D], F32, kind="ExternalInput").ap()
    out_d = nc.dram_tensor("out", [BN, D], F32, kind="ExternalOutput").ap()
    scr_d = nc.dram_tensor("scr", [128, 8], F8)

    with tile.TileContext(nc) as tc:
        with (
            tc.tile_pool(name="const", bufs=1) as constp,
            tc.tile_pool(name="xs", bufs=3) as xsp,
            tc.tile_pool(name="ut", bufs=6) as utp,
            tc.tile_pool(name="u0t", bufs=2) as u0p,
            tc.tile_pool(name="ost", bufs=3) as ostp,
            tc.tile_pool(name="psU0", bufs=2, space="PSUM") as psU0,
            tc.tile_pool(name="psU", bufs=3, space="PSUM") as psU,
            tc.tile_pool(name="psH", bufs=3, space="PSUM") as psH,
        ):
            # ---- persistent inputs ----
            # gpsimd carries the startup-critical fp8 stream (X8 rt-major
            # chunks in consumption order + V8); sync/scalar queues are
            # gated behind it with probe DMAs so U0/S8 inputs don't steal
            # bandwidth before the first projection phases are fed.
            X8 = constp.tile([128, BN // 128, DT, 128], F8, name="X8")
            V8s = constp.tile([128, 3, DT, D], F8, name="V8s")
            nc.gpsimd.dma_start(X8[:, 0, :, :], x8P_d[0, :, :, :])
            nc.gpsimd.dma_start(X8[:, 1, :, :], x8P_d[1, :, :, :])
            nc.gpsimd.dma_start(V8s[:, 2, :, :], v8P_d[2, :, :, :])
            for rt in range(2, 8):
                nc.gpsimd.dma_start(X8[:, rt, :, :], x8P_d[rt, :, :, :])
            nc.gpsimd.dma_start(V8s[:, 1, :, :], v8P_d[1, :, :, :])
            nc.gpsimd.dma_start(V8s[:, 0, :, :], v8P_d[0, :, :, :])
            biasb = constp.tile([128, D], F32, name="biasb")
            nc.gpsimd.dma_start(
                biasb[:], bias_d.unsqueeze(0).broadcast_to([128, D]))
            for rt in range(8, BN // 128):
                nc.gpsimd.dma_start(X8[:, rt, :, :], x8P_d[rt, :, :, :])
            nc.sync.dma_start(scr_d[:, 0:4], X8[:, 7, 0, 0:4])
            V0s = constp.tile([128, DT, D], BF16, name="V0s")
            for t in range(DT):
                nc.sync.dma_start(V0s[:, t, :], v0P_d[:, t, :])
            nc.scalar.dma_start(scr_d[:, 4:8], V8s[:, 0, 0, 0:4])
            S8s = constp.tile([128, 3, NT, N], F8, name="S8s")
            for pm in range(3):
                nc.scalar.dma_start(S8s[:, pm, :, :], s8P_d[pm, :, :, :])

            # PE warm-up: junk matmuls during the initial DMA wait ramp the
            # tensor-engine p-state so real work starts at full clock.
            warm = constp.tile([128, 512], BF16, name="warm")
            nc.vector.memset(warm[:], 0.0)
            for i in range(20):
                pw = psU0.tile([128, 512], F32, name=f"warm{i}", tag="ps")
                nc.tensor.matmul(pw[:], warm[:, 0:128], warm[:],
                                 start=True, stop=True)

            def proj_u0(pair):
                """U0 (+bias) for batch pair (bf16 matmuls) -> U0b fp32."""
                u0b = u0p.tile([128, NT, PW], F32, name=f"u0b{pair}",
                               tag="u0")
                for bi in range(2):
                    b = 2 * pair + bi
                    for nt in range(NT):
                        rt = b * NT + nt
                        xt = xsp.tile([128, DT, 128], BF16,
                                      name=f"xt{rt}", tag="xt")
                        nc.sync.dma_start(xt[:], x0R_d[rt, :, :, :])
                        ps = psU0.tile([128, 512], F32, name=f"pA{rt}",
                                       tag="ps")
                        ps2 = psU0.tile([128, 512], F32, name=f"pB{rt}",
                                        tag="ps")
                        # t outer: both column groups share each stationary
                        for t in range(DT):
                            nc.tensor.matmul(
                                ps[:], xt[:, t, :], V0s[:, t, 0:512],
                                start=(t == 0), stop=(t == DT - 1))
                            nc.tensor.matmul(
                                ps2[:, 0:256], xt[:, t, :], V0s[:, t, 512:D],
                                start=(t == 0), stop=(t == DT - 1))
                        c0 = bi * D
                        nc.vector.tensor_add(
                            u0b[:, nt, c0:c0 + 512], ps[:], biasb[:, 0:512])
                        nc.vector.tensor_add(
                            u0b[:, nt, c0 + 512:c0 + D], ps2[:, 0:256],
                            biasb[:, 512:D])
                return u0b

            def proj_um(pair, m, dst):
                """One U_m phase (fp8 DoubleRow) for a batch pair -> fp8."""
                for bi in range(2):
                    b = 2 * pair + bi
                    for nt in range(NT):
                        rt = b * NT + nt
                        c0 = bi * D
                        ps = psU.tile([128, 512], F32,
                                      name=f"pU{rt}_{m}", tag="ps")
                        ps2 = psU.tile([128, 512], F32,
                                       name=f"pV{rt}_{m}", tag="ps")
                        # t outer: both column groups share the stationary
                        for t in range(DT // 2):
                            xs = X8[:, rt, 2 * t:2 * t + 2, :]
                            nc.tensor.matmul(
                                ps[:], xs,
                                V8s[:, m - 1, 2 * t:2 * t + 2, 0:512],
                                start=(t == 0), stop=(t == 2),
                                perf_mode=DR)
                            nc.tensor.matmul(
                                ps2[:, 0:256], xs,
                                V8s[:, m - 1, 2 * t:2 * t + 2, 512:D],
                                start=(t == 0), stop=(t == 2),
                                perf_mode=DR)
                        nc.scalar.activation(
                            dst[:, nt, c0:c0 + 512], ps[:],
                            AFT.Copy, scale=C_U)
                        nc.scalar.activation(
                            dst[:, nt, c0 + 512:c0 + D], ps2[:, 0:256],
                            AFT.Copy, scale=C_U)

            def combine(pair, u8, u0b):
                """out = U0b + sum_m S^m @ U_m; one PSUM group per tile."""
                for nt in range(NT):
                    ot = ostp.tile([128, PW], F32, name=f"o{pair}_{nt}",
                                   tag="ost")
                    for ck in range(PW // 512):
                        ps = psH.tile([128, 512], F32,
                                      name=f"pH{pair}{nt}{ck}", tag="ps")
                        first = True
                        for pm in range(3):
                            for t in range(NT // 2):
                                nc.tensor.matmul(
                                    ps[:],
                                    S8s[:, pm, 2 * t:2 * t + 2,
                                        nt * 128:(nt + 1) * 128],
                                    u8[pm][:, 2 * t:2 * t + 2,
                                           ck * 512:(ck + 1) * 512],
                                    start=first,
                                    stop=(pm == 2 and t == 1),
                                    perf_mode=DR)
                                first = False
                        nc.vector.scalar_tensor_tensor(
                            ot[:, ck * 512:(ck + 1) * 512],
                            ps[:], C_O,
                            u0b[:, nt, ck * 512:(ck + 1) * 512],
                            ALU.mult, ALU.add)
                    for bi in range(2):
                        b = 2 * pair + bi
                        r0 = b * N + nt * 128
                        eng = (nc.gpsimd, nc.sync, nc.scalar)[
                            (2 * nt + bi) % 3]
                        eng.dma_start(
                            out_d[r0:r0 + 128, :],
                            ot[:, bi * D:(bi + 1) * D])

            # ---- schedule ----
            def mk_u8(pair):
                # index pm: 0 -> U1, 1 -> U2, 2 -> U3
                return [utp.tile([128, NT, PW], F8,
                                 name=f"u8_{pair}_{pm}", tag="u")
                        for pm in range(3)]

            u8_0, u8_1 = mk_u8(0), mk_u8(1)

            proj_um(0, 3, u8_0[2])
            proj_um(0, 2, u8_0[1])
            proj_um(0, 1, u8_0[0])
            u0b_0 = proj_u0(0)

            combine(0, u8_0, u0b_0)

            proj_um(1, 3, u8_1[2])
            proj_um(1, 2, u8_1[1])
            proj_um(1, 1, u8_1[0])
            u0b_1 = proj_u0(1)

            combine(1, u8_1, u0b_1)
    nc.compile()
    return nc


_CACHE = {}


def _get_program():
    if "nc" not in _CACHE:
        _CACHE["nc"] = _build_program()
    return _CACHE["nc"]


def make_in_maps(inputs, adj, weights, biases):
    inputs = np.ascontiguousarray(inputs, dtype=np.float32)
    adj = np.ascontiguousarray(adj, dtype=np.float32)
    weights = np.ascontiguousarray(weights, dtype=np.float32)
    biases = np.ascontiguousarray(biases, dtype=np.float32)
    assert inputs.shape == (B, N, D)
    assert adj.shape == (N, N)
    assert weights.shape == (D * 4, D)
    assert biases.shape == (D,)

    def perm(a, kt):
        # [kt*128, F] -> [128, kt, F] partition-major tile image
        F = a.shape[1]
        return np.ascontiguousarray(
            a.reshape(kt, 128, F).transpose(1, 0, 2))

    # support S = D^-1/2 (adj+I)^T D^-1/2; powers in fp32, then fp8.
    # lhsT layout needs (S^m)^T = (S^T)^m.
    m = adj + np.eye(N, dtype=np.float32)
    dd = m.sum(axis=1) ** -0.5
    ST = np.ascontiguousarray(
        ((m * dd[None, :]).T * dd[None, :]).astype(np.float32).T)
    ST2 = (ST @ ST).astype(np.float32)
    ST3 = (ST2 @ ST).astype(np.float32)
    s8P = np.ascontiguousarray(np.stack(
        [perm((p * SS).astype(E4M3), NT) for p in (ST, ST2, ST3)]))

    W4 = weights.reshape(D, 4, D)
    v0P = perm(np.ascontiguousarray(W4[:, 0] - W4[:, 2])
               .astype(ml_dtypes.bfloat16), DT)
    v8P = np.ascontiguousarray(np.stack([
        perm(np.ascontiguousarray(v * SV).astype(E4M3), DT)
        for v in (W4[:, 1] - 3.0 * W4[:, 3], 2.0 * W4[:, 2],
                  4.0 * W4[:, 3])]))

    in_maps = []
    for c in range(N_CORES):
        xc = inputs[c * BL:(c + 1) * BL].reshape(BN, D)
        # x0R[rt, p, t, r] = xc[rt*128+r, t*128+p]
        x0R = np.ascontiguousarray(
            xc.reshape(BN // 128, 128, DT, 128).transpose(0, 3, 2, 1)
            .astype(ml_dtypes.bfloat16))
        # x8P[rt, p, t, r] = 8*xc[rt*128+r, t*128+p] quantized
        x8P = np.ascontiguousarray(
            (xc * SX).reshape(BN // 128, 128, DT, 128)
            .transpose(0, 3, 2, 1).astype(E4M3))
        in_maps.append({
            "x0R": x0R,
            "x8P": x8P,
            "v0P": v0P,
            "v8P": v8P,
            "s8P": s8P,
            "bias": biases,
        })
    return in_maps


def kernel(inputs, adj, weights, biases):
    nc = _get_program()
    in_maps = make_in_maps(inputs, adj, weights, biases)
    res = run_bass_kernel_spmd(nc, in_maps, list(range(N_CORES)))
    out = np.concatenate(
        [res.results[c]["out"].reshape(BL, N, D) for c in range(N_CORES)],
        axis=0)
    return out


# revision 29
# speedup vs baseline: 1.0097x; 1.0097x over previous
"""DGCN diffusion-graph-conv kernel for 8 Trainium2 NeuronCores.

Math (per the reference):
    support S = D^-1/2 (adj+I)^T D^-1/2  with D = diag(rowsum(adj+I))
    x_m = T_m(S) x0  (Chebyshev recurrence, K=3 -> m=0..3)
    out = sum_m x_m @ W_m + bias

Strategy (data-parallel over batch, 4 batches/core, mixed precision):
    Fold Chebyshev coefficients into the weights and expand the
    recurrence into explicit support powers (host-precomputed in fp32):
        V0 = W0 - W2, V1 = W1 - 3*W3, V2 = 2*W2, V3 = 4*W3
        U_m = x0 @ V_m                        (contracts feature dim)
        out = U0 + S*U1 + S^2*U2 + S^3*U3 + bias   (contracts nodes)
    U0 feeds the output undamped -> bf16 matmuls (full-rate, ~0.2% err).
    U1..U3 and the S^m multiplies are damped by the contractive support
    spectrum -> fp8 e4m3 DoubleRow matmuls (2x PE rate).  The three
    S^m terms accumulate into one PSUM group, so there is no serial
    diffusion chain on the device at all.
    Power-of-2 scales keep fp8 operands in the normal range:
        x*8, V_m*32 (m>=1), S^m*2^14, U_m carried *8.
    Host precomputes S, S^2, S^3, the V combos, all layout permutes and
    fp8 quantization.  Measured end-to-end rel err ~9e-3 (gate 2e-2).
"""

import numpy as np
import ml_dtypes

import concourse.bacc as bacc
import concourse.tile as tile
import concourse.mybir as mybir
from concourse.bass_utils import run_bass_kernel_spmd

F32 = mybir.dt.float32
BF16 = mybir.dt.bfloat16
F8 = mybir.dt.float8e4
DR = mybir.MatmulPerfMode.DoubleRow
ALU = mybir.AluOpType
AFT = mybir.ActivationFunctionType
E4M3 = ml_dtypes.float8_e4m3

N_CORES = 8
B, N, D = 32, 512, 768
BL = B // N_CORES          # local batches per core = 4
BN = BL * N                # local rows = 2048
NT = N // 128              # 4 node tiles per batch
DT = D // 128              # 6 feature k-subtiles
PW = 1536                  # batch-pair column width (2*768)

SX = 8.0                   # x fp8 pre-scale
SV = 32.0                  # V1..V3 fp8 pre-scale
SS = float(2 ** 14)        # S^m fp8 pre-scale
SU = 8.0                   # U_m carried *8 in fp8
C_U = SU / (SX * SV)       # psum(U123) -> 8*U_m     (= 1/32)
C_O = 1.0 / (SS * SU)      # psum(combine) -> out    (= 2^-17)


def _build_program():
    nc = bacc.Bacc("TRN2", target_bir_lowering=False, debug=False,
                   num_devices=N_CORES)
    # All inputs host-permuted to the exact SBUF tile layout
    # ([partition, ...free]) so every DMA moves full contiguous lines.
    x0R_d = nc.dram_tensor("x0R", [BN // 128, 128, DT, 128], BF16,
                           kind="ExternalInput").ap()
    x8P_d = nc.dram_tensor("x8P", [BN // 128, 128, DT, 128], F8,
                           kind="ExternalInput").ap()
    v0P_d = nc.dram_tensor("v0P", [128, DT, D], BF16,
                           kind="ExternalInput").ap()
    v8P_d = nc.dram_tensor("v8P", [3, 128, DT, D], F8,
                           kind="ExternalInput").ap()
    s8P_d = nc.dram_tensor("s8P", [3, 128, NT, N], F8,
                           kind="ExternalInput").ap()
    bias_d = nc.dram_tensor("bias", [D], F32, kind="ExternalInput").ap()
    out_d = nc.dram_tensor("out", [BN, D], F32, kind="ExternalOutput").ap()

    with tile.TileContext(nc) as tc:
        with (
            tc.tile_pool(name="const", bufs=1) as constp,
            tc.tile_pool(name="xs", bufs=3) as xsp,
            tc.tile_pool(name="ut", bufs=6) as utp,
            tc.tile_pool(name="u0t", bufs=2) as u0p,
            tc.tile_pool(name="ost", bufs=3) as ostp,
            tc.tile_pool(name="psU0", bufs=2, space="PSUM") as psU0,
            tc.tile_pool(name="psU", bufs=3, space="PSUM") as psU,
            tc.tile_pool(name="psH", bufs=3, space="PSUM") as psH,
        ):
            # ---- persistent inputs, startup-critical first ----
            V0s = constp.tile([128, DT, D], BF16, name="V0s")
            for t in range(DT):
                eng = (nc.sync, nc.sync, nc.scalar,
                       nc.scalar, nc.gpsimd, nc.gpsimd)[t]
                eng.dma_start(V0s[:, t, :], v0P_d[:, t, :])
            # X8 rt-major: per-row-tile chunks land in consumption order,
            # so the first fp8 group starts as soon as ~0.7MB has arrived
            X8 = constp.tile([128, BN // 128, DT, 128], F8, name="X8")
            V8s = constp.tile([128, 3, DT, D], F8, name="V8s")
            nc.gpsimd.dma_start(X8[:, 0, :, :], x8P_d[0, :, :, :])
            nc.gpsimd.dma_start(X8[:, 1, :, :], x8P_d[1, :, :, :])
            nc.gpsimd.dma_start(V8s[:, 2, :, :], v8P_d[2, :, :, :])
            for rt in range(2, 8):
                nc.gpsimd.dma_start(X8[:, rt, :, :], x8P_d[rt, :, :, :])
            nc.gpsimd.dma_start(V8s[:, 1, :, :], v8P_d[1, :, :, :])
            nc.gpsimd.dma_start(V8s[:, 0, :, :], v8P_d[0, :, :, :])
            biasb = constp.tile([128, D], F32, name="biasb")
            nc.gpsimd.dma_start(
                biasb[:], bias_d.unsqueeze(0).broadcast_to([128, D]))
            for rt in range(8, BN // 128):
                nc.gpsimd.dma_start(X8[:, rt, :, :], x8P_d[rt, :, :, :])
            S8s = constp.tile([128, 3, NT, N], F8, name="S8s")
            for pm in range(3):
                nc.scalar.dma_start(S8s[:, pm, :, :], s8P_d[pm, :, :, :])

            # PE warm-up: junk matmuls during the initial DMA wait ramp the
            # tensor-engine p-state so real work starts at full clock.
            warm = constp.tile([128, 512], BF16, name="warm")
            nc.vector.memset(warm[:], 0.0)
            for i in range(20):
                pw = psU0.tile([128, 512], F32, name=f"warm{i}", tag="ps")
                nc.tensor.matmul(pw[:], warm[:, 0:128], warm[:],
                                 start=True, stop=True)

            def proj_u0(pair):
                """U0 (+bias) for batch pair (bf16 matmuls) -> U0b fp32."""
                u0b = u0p.tile([128, NT, PW], F32, name=f"u0b{pair}",
                               tag="u0")
                for bi in range(2):
                    b = 2 * pair + bi
                    for nt in range(NT):
                        rt = b * NT + nt
                        xt = xsp.tile([128, DT, 128], BF16,
                                      name=f"xt{rt}", tag="xt")
                        nc.sync.dma_start(xt[:], x0R_d[rt, :, :, :])
                        ps = psU0.tile([128, 512], F32, name=f"pA{rt}",
                                       tag="ps")
                        ps2 = psU0.tile([128, 512], F32, name=f"pB{rt}",
                                        tag="ps")
                        # t outer: both column groups share each stationary
                        for t in range(DT):
                            nc.tensor.matmul(
                                ps[:], xt[:, t, :], V0s[:, t, 0:512],
                                start=(t == 0), stop=(t == DT - 1))
                            nc.tensor.matmul(
                                ps2[:, 0:256], xt[:, t, :], V0s[:, t, 512:D],
                                start=(t == 0), stop=(t == DT - 1))
                        c0 = bi * D
                        nc.vector.tensor_add(
                            u0b[:, nt, c0:c0 + 512], ps[:], biasb[:, 0:512])
                        nc.vector.tensor_add(
                            u0b[:, nt, c0 + 512:c0 + D], ps2[:, 0:256],
                            biasb[:, 512:D])
                return u0b

            def proj_um(pair, m, dst):
                """One U_m phase (fp8 DoubleRow) for a batch pair -> fp8."""
                for bi in range(2):
                    b = 2 * pair + bi
                    for nt in range(NT):
                        rt = b * NT + nt
                        c0 = bi * D
                        ps = psU.tile([128, 512], F32,
                                      name=f"pU{rt}_{m}", tag="ps")
                        ps2 = psU.tile([128, 512], F32,
                                       name=f"pV{rt}_{m}", tag="ps")
                        # t outer: both column groups share the stationary
                        for t in range(DT // 2):
                            xs = X8[:, rt, 2 * t:2 * t + 2, :]
                            nc.tensor.matmul(
                                ps[:], xs,
                                V8s[:, m - 1, 2 * t:2 * t + 2, 0:512],
                                start=(t == 0), stop=(t == 2),
                                perf_mode=DR)
                            nc.tensor.matmul(
                                ps2[:, 0:256], xs,
                                V8s[:, m - 1, 2 * t:2 * t + 2, 512:D],
                                start=(t == 0), stop=(t == 2),
                                perf_mode=DR)
                        nc.scalar.activation(
                            dst[:, nt, c0:c0 + 512], ps[:],
                            AFT.Copy, scale=C_U)
                        nc.scalar.activation(
                            dst[:, nt, c0 + 512:c0 + D], ps2[:, 0:256],
                            AFT.Copy, scale=C_U)

            def combine(pair, u8, u0b):
                """out = U0b + sum_m S^m @ U_m; one PSUM group per tile."""
                for nt in range(NT):
                    ot = ostp.tile([128, PW], F32, name=f"o{pair}_{nt}",
                                   tag="ost")
                    for ck in range(PW // 512):
                        ps = psH.tile([128, 512], F32,
                                      name=f"pH{pair}{nt}{ck}", tag="ps")
                        first = True
                        for pm in range(3):
                            for t in range(NT // 2):
                                nc.tensor.matmul(
                                    ps[:],
                                    S8s[:, pm, 2 * t:2 * t + 2,
                                        nt * 128:(nt + 1) * 128],
                                    u8[pm][:, 2 * t:2 * t + 2,
                                           ck * 512:(ck + 1) * 512],
                                    start=first,
                                    stop=(pm == 2 and t == 1),
                                    perf_mode=DR)
                                first = False
                        nc.vector.scalar_tensor_tensor(
                            ot[:, ck * 512:(ck + 1) * 512],
                            ps[:], C_O,
                            u0b[:, nt, ck * 512:(ck + 1) * 512],
                            ALU.mult, ALU.add)
                    for bi in range(2):
                        b = 2 * pair + bi
                        r0 = b * N + nt * 128
                        eng = (nc.gpsimd, nc.sync, nc.scalar)[
                            (2 * nt + bi) % 3]
                        eng.dma_start(
                            out_d[r0:r0 + 128, :],
                            ot[:, bi * D:(bi + 1) * D])

            # ---- schedule ----
            def mk_u8(pair):
                # index pm: 0 -> U1, 1 -> U2, 2 -> U3
                return [utp.tile([128, NT, PW], F8,
                                 name=f"u8_{pair}_{pm}", tag="u")
                        for pm in range(3)]

            u8_0, u8_1 = mk_u8(0), mk_u8(1)

            proj_um(0, 3, u8_0[2])
            proj_um(0, 2, u8_0[1])
            proj_um(0, 1, u8_0[0])
            u0b_0 = proj_u0(0)

            combine(0, u8_0, u0b_0)

            proj_um(1, 3, u8_1[2])
            proj_um(1, 2, u8_1[1])
            proj_um(1, 1, u8_1[0])
            u0b_1 = proj_u0(1)

            combine(1, u8_1, u0b_1)
    nc.compile()
    return nc


_CACHE = {}


def _get_program():
    if "nc" not in _CACHE:
        _CACHE["nc"] = _build_program()
    return _CACHE["nc"]


def make_in_maps(inputs, adj, weights, biases):
    inputs = np.ascontiguousarray(inputs, dtype=np.float32)
    adj = np.ascontiguousarray(adj, dtype=np.float32)
    weights = np.ascontiguousarray(weights, dtype=np.float32)
    biases = np.ascontiguousarray(biases, dtype=np.float32)
    assert inputs.shape == (B, N, D)
    assert adj.shape == (N, N)
    assert weights.shape == (D * 4, D)
    assert biases.shape == (D,)

    def perm(a, kt):
        # [kt*128, F] -> [128, kt, F] partition-major tile image
        F = a.shape[1]
        return np.ascontiguousarray(
            a.reshape(kt, 128, F).transpose(1, 0, 2))

    # support S = D^-1/2 (adj+I)^T D^-1/2; powers in fp32, then fp8.
    # lhsT layout needs (S^m)^T = (S^T)^m.
    m = adj + np.eye(N, dtype=np.float32)
    dd = m.sum(axis=1) ** -0.5
    ST = np.ascontiguousarray(
        ((m * dd[None, :]).T * dd[None, :]).astype(np.float32).T)
    ST2 = (ST @ ST).astype(np.float32)
    ST3 = (ST2 @ ST).astype(np.float32)
    s8P = np.ascontiguousarray(np.stack(
        [perm((p * SS).astype(E4M3), NT) for p in (ST, ST2, ST3)]))

    W4 = weights.reshape(D, 4, D)
    v0P = perm(np.ascontiguousarray(W4[:, 0] - W4[:, 2])
               .astype(ml_dtypes.bfloat16), DT)
    v8P = np.ascontiguousarray(np.stack([
        perm(np.ascontiguousarray(v * SV).astype(E4M3), DT)
        for v in (W4[:, 1] - 3.0 * W4[:, 3], 2.0 * W4[:, 2],
                  4.0 * W4[:, 3])]))

    in_maps = []
    for c in range(N_CORES):
        xc = inputs[c * BL:(c + 1) * BL].reshape(BN, D)
        # x0R[rt, p, t, r] = xc[rt*128+r, t*128+p]
        x0R = np.ascontiguousarray(
            xc.reshape(BN // 128, 128, DT, 128).transpose(0, 3, 2, 1)
            .astype(ml_dtypes.bfloat16))
        # x8P[rt, p, t, r] = 8*xc[rt*128+r, t*128+p] quantized
        x8P = np.ascontiguousarray(
            (xc * SX).reshape(BN // 128, 128, DT, 128)
            .transpose(0, 3, 2, 1).astype(E4M3))
        in_maps.append({
            "x0R": x0R,
            "x8P": x8P,
            "v0P": v0P,
            "v8P": v8P,
            "s8P": s8P,
            "bias": biases,
        })
    return in_maps


def kernel(inputs, adj, weights, biases):
    nc = _get_program()
    in_maps = make_in_maps(inputs, adj, weights, biases)
    res = run_bass_kernel_spmd(nc, in_maps, list(range(N_CORES)))
    out = np.concatenate(
        [res.results[c]["out"].reshape(BL, N, D) for c in range(N_CORES)],
        axis=0)
    return out
